# revision 28
# baseline (speedup 1.0000x reference)
"""Trainium2 Bass kernel for nn_ArmInt_19911468384433 (dense_mlp, 8 cores).

Data-parallel: x [2097152, 32] sharded by rows across 8 NeuronCores; tiny
32x32 weights folded/replicated. All math (3 integer-MLP layers with
emulated fixed-point rounding, exp/clip head) runs on device. Host does
layout permutation (shard + transpose-pack of input, fp16 hi/lo split,
inverse reshape of output) and weight folding only.

Device algorithm per core (S = 262144 rows = 128 tiles of 2048 rows):
  x pre-packed on host as xd*[t, 32b+c, f'] = x[2048 t + 512 b + f', c],
  split x = xhi (fp16) + xlo (fp8e4m3): ~15-bit combined mantissa, 3B/elem
  input DMA; boundary-flip rate stays within the 2e-2 error budget.
  Per tile:
    mm1 : ps1 = W1s.T @ xhi + W1s.T @ xlo   (fp16 stationary; fp8 moving
          for the lo half — mixed-dtype matmul, PSUM f32 accumulate)
    ep1 : t = Relu(ps1 + bc1) (ACT, bias AP)  then h1 = rne(t) via DVE
          fused ts (+2^23, -2^23) -> fp16   [or the all-DVE form below]
    mm2 : ps2 = W2s.T @ h1  (fp16);  ep2 likewise -> h2
    mm3 : ps3pack += W3s_tau.T @ h2  (16 tiles accumulate into one bank)
  Every 4th layer-slot uses the all-DVE form to balance engines:
    u = ts(ps + bc, + 2^23) ; h = ts(u - 2^23, max 0) -> fp16
  Per 16-tile pack (DVE except Exp):
    t3 = ps3pack + bc3 ; y = rne(t3) ; outA = y/256
    e = Exp(y/256 - 4) (ACT) ; outB = clip(e, e^-4.6, e^5)

rne(v + 2^-9) == trunc-round-half-away-from-zero on the 1/256-granular
values here; verified vs reference at rel err ~5e-3 (fp32 order noise).
"""
import sys

sys.path.insert(0, "/opt/trn_rl_repo")

from contextlib import ExitStack

import numpy as np

import concourse.bacc as bacc
import concourse.bass as bass
import concourse.tile as tile
from concourse import mybir
from concourse.bass_utils import run_bass_kernel_spmd

F32 = mybir.dt.float32
F16 = mybir.dt.float16
F8 = mybir.dt.float8e4
AF = mybir.ActivationFunctionType
ALU = mybir.AluOpType

B = 2097152
C = 32
NCORES = 8
S = B // NCORES            # 262144 rows per core
NT = S // 2048             # 128 tiles per core
NPACK = NT // 16           # 8 packs per core
C2 = float(2.0 ** 23)
CTIE = float(2.0 ** -9)

_compiled = {}


def _layer_ep(nc, pools, ps, bc_sb, bcm_sb, out_dt, dve_form, tagp):
    """PSUM -> relu(rne(v + bc)) -> SBUF tile (out_dt)."""
    tpool, hpool = pools
    if not dve_form:
        t = tpool.tile([128, 512], F32, tag=tagp + "t", name="t")
        nc.scalar.activation(t, ps, AF.Relu, bias=bc_sb, scale=1.0)
        h = hpool.tile([128, 512], out_dt, tag=tagp + "h", name="h")
        nc.vector.tensor_scalar(h, t, C2, C2, ALU.add, ALU.subtract)
    else:
        u = tpool.tile([128, 512], F32, tag=tagp + "t", name="u")
        nc.vector.tensor_scalar(u, ps, bc_sb, C2, ALU.add, ALU.add)
        h = hpool.tile([128, 512], out_dt, tag=tagp + "h", name="h")
        nc.vector.tensor_scalar(h, u, C2, 0.0, ALU.subtract, ALU.max)
    return h


def _build_graph():
    nc = bacc.Bacc("TRN2", target_bir_lowering=False, debug=False)
    # x split: hi fp16 + lo fp8e4m3 (exact-enough 15-bit x; mixed-dtype
    # matmul fp16 stationary x fp8 moving verified exact on HW).
    # 4 tiles per DMA group: xhi [128, 4*512] fp16, xlo [128, 4*512] fp8.
    xhi = nc.declare_dram_parameter("xhi", [NT // 4, 128, 2048], F16, isOutput=False)
    xlo = nc.declare_dram_parameter("xlo", [NT // 4, 128, 2048], F8, isOutput=False)
    w1s = nc.declare_dram_parameter("w1s", [128, 128], F16, isOutput=False)
    w2s = nc.declare_dram_parameter("w2s", [128, 128], F16, isOutput=False)
    w3s = nc.declare_dram_parameter("w3s", [128, 2048], F16, isOutput=False)
    bc1 = nc.declare_dram_parameter("bc1", [128, 1], F32, isOutput=False)
    bc2 = nc.declare_dram_parameter("bc2", [128, 1], F32, isOutput=False)
    bc3 = nc.declare_dram_parameter("bc3", [128, 1], F32, isOutput=False)
    outa = nc.declare_dram_parameter("outa", [NPACK, 128, 512], F16, isOutput=True)
    outb = nc.declare_dram_parameter("outb", [NPACK, 128, 512], F16, isOutput=True)

    with ExitStack() as ctx:
        tc = ctx.enter_context(tile.TileContext(nc))
        consts = ctx.enter_context(tc.tile_pool(name="consts", bufs=1))
        xpool = ctx.enter_context(tc.tile_pool(name="xpool", bufs=5))
        tpool = ctx.enter_context(tc.tile_pool(name="tpool", bufs=4))
        hpool = ctx.enter_context(tc.tile_pool(name="hpool", bufs=4))
        opool = ctx.enter_context(tc.tile_pool(name="opool", bufs=3))
        ps1p = ctx.enter_context(tc.tile_pool(name="ps1p", bufs=2, space="PSUM"))
        ps2p = ctx.enter_context(tc.tile_pool(name="ps2p", bufs=2, space="PSUM"))
        ps3p = ctx.enter_context(tc.tile_pool(name="ps3p", bufs=2, space="PSUM"))

        w1_sb = consts.tile([128, 128], F16, tag="w1", name="w1_sb")
        nc.gpsimd.dma_start(out=w1_sb, in_=w1s[:])
        w2_sb = consts.tile([128, 128], F16, tag="w2", name="w2_sb")
        nc.gpsimd.dma_start(out=w2_sb, in_=w2s[:])
        w3_sb = consts.tile([128, 2048], F16, tag="w3", name="w3_sb")
        nc.gpsimd.dma_start(out=w3_sb, in_=w3s[:])
        bc1_sb = consts.tile([128, 1], F32, tag="bc1", name="bc1_sb")
        nc.gpsimd.dma_start(out=bc1_sb, in_=bc1[:])
        bc2_sb = consts.tile([128, 1], F32, tag="bc2", name="bc2_sb")
        nc.gpsimd.dma_start(out=bc2_sb, in_=bc2[:])
        bc3_sb = consts.tile([128, 1], F32, tag="bc3", name="bc3_sb")
        nc.gpsimd.dma_start(out=bc3_sb, in_=bc3[:])
        bm4_sb = consts.tile([128, 1], F32, tag="bm4", name="bm4_sb")
        nc.vector.memset(bm4_sb, -4.0)
        c2_sb = consts.tile([128, 1], F32, tag="c2c", name="c2_sb")
        nc.vector.memset(c2_sb, C2)

        for pack in range(NPACK):
            ps3 = ps3p.tile([128, 512], F32, tag="ps3", name="ps3")
            for tau in range(16):
                t = pack * 16 + tau
                if tau % 4 == 0:
                    xh4 = xpool.tile([128, 2048], F16, tag="xh4", name="xh4")
                    xl4 = xpool.tile([128, 2048], F8, tag="xl4", name="xl4")
                    nc.sync.dma_start(out=xh4, in_=xhi[t // 4])
                    nc.gpsimd.dma_start(out=xl4, in_=xlo[t // 4])
                if tau % 2 == 0:
                    t1b = tpool.tile([128, 1024], F32, tag="t1b", name="t1b")
                    h1b = hpool.tile([128, 1024], F16, tag="h1b", name="h1b")
                off = 512 * (tau % 4)

                # layer 1: per-tile 1-bank PSUM; evacs into halves of t1b,
                # one batched rne per 2 tiles
                ps1 = ps1p.tile([128, 512], F32, tag="ps1", name="ps1")
                nc.tensor.matmul(ps1, w1_sb, xh4[:, off:off + 512],
                                 start=True, stop=False)
                nc.tensor.matmul(ps1, w1_sb, xl4[:, off:off + 512],
                                 start=False, stop=True)
                half = slice(512 * (tau % 2), 512 * (tau % 2) + 512)
                nc.scalar.activation(t1b[:, half], ps1, AF.Relu, bias=bc1_sb,
                                     scale=1.0)
                if tau % 2 == 1:
                    nc.vector.tensor_scalar(h1b, t1b, C2, C2, ALU.add,
                                            ALU.subtract)

                # layer 2 for the two tiles of the group (after h1b ready).
                # Pattern over 4 tiles: [A A D D] — A-pairs share one batched
                # rne over [128, 1024]; D uses the all-DVE per-tile form.
                if tau % 2 == 1:
                    act_pair = (t % 4) == 1 or (t % 16) == 15
                    if act_pair:
                        t2b = tpool.tile([128, 1024], F32, tag="t2b", name="t2b")
                        h2b = hpool.tile([128, 1024], F16, tag="h2b", name="h2b")
                    h2s = []
                    for k, tprev in enumerate((t - 1, t)):
                        h1 = h1b[:, 512 * (tprev % 2):512 * (tprev % 2) + 512]
                        ps2 = ps2p.tile([128, 512], F32, tag="ps2", name="ps2")
                        nc.tensor.matmul(ps2, w2_sb, h1, start=True, stop=True)
                        if act_pair:
                            nc.scalar.activation(t2b[:, 512 * k:512 * k + 512],
                                                 ps2, AF.Relu, bias=bc2_sb,
                                                 scale=1.0)
                            h2s.append(None)
                        else:
                            h2s.append(_layer_ep(nc, (tpool, hpool), ps2,
                                                 bc2_sb, bm4_sb, F16,
                                                 dve_form=True, tagp="l2"))
                    if act_pair:
                        nc.vector.tensor_scalar(h2b, t2b, C2, C2, ALU.add,
                                                ALU.subtract)
                        h2s = [h2b[:, 0:512], h2b[:, 512:1024]]
                    for k, tprev in enumerate((t - 1, t)):
                        tau_p = tprev - pack * 16
                        nc.tensor.matmul(
                            ps3, w3_sb[:, 128 * tau_p:128 * (tau_p + 1)],
                            h2s[k], start=(tau_p == 0), stop=(tau_p == 15))

            t3 = tpool.tile([128, 512], F32, tag="t3", name="t3")
            nc.vector.tensor_scalar(t3, ps3, bc3_sb, C2, ALU.add, ALU.add)
            y = tpool.tile([128, 512], F32, tag="y", name="y")
            nc.vector.tensor_scalar(y, t3, C2, None, ALU.subtract)
            oa = opool.tile([128, 512], F16, tag="oa", name="oa")
            nc.vector.tensor_scalar(oa, y, 1.0 / 256.0, None, ALU.mult)
            e = tpool.tile([128, 512], F32, tag="e", name="e")
            nc.scalar.activation(e, y, AF.Exp, bias=bm4_sb, scale=1.0 / 256.0)
            ob = opool.tile([128, 512], F16, tag="ob", name="ob")
            nc.vector.tensor_scalar(ob, e, float(np.exp(5.0)),
                                    float(np.exp(-4.6)), ALU.min, ALU.max)
            nc.sync.dma_start(out=outa[pack], in_=oa)
            nc.sync.dma_start(out=outb[pack], in_=ob)

    nc.compile()
    return nc


def _get_graph():
    if "nc" not in _compiled:
        _compiled["nc"] = _build_graph()
    return _compiled["nc"]


def _prep_weights(w0, b0, w1, b1, w_out, b_out):
    eye = np.eye(C, dtype=np.float32)
    W0s = (w0.T.astype(np.float32) + 256.0 * eye).astype(np.float16)
    W1s_small = ((w1.T.astype(np.float32) + 256.0 * eye) / 256.0).astype(np.float16)
    W3_small = (w_out.T.astype(np.float32) / 256.0).astype(np.float16)  # [32, 2]

    w1s = np.zeros((128, 128), np.float16)
    w2s = np.zeros((128, 128), np.float16)
    for b in range(4):
        w1s[32 * b:32 * b + 32, 32 * b:32 * b + 32] = W0s
        w2s[32 * b:32 * b + 32, 32 * b:32 * b + 32] = W1s_small

    # mm3 stationary for inner-loop index tau: out partition m = 8 tau + 4 o + b
    w3pack = np.zeros((16, 128, 128), np.float16)
    for tau in range(16):
        for b in range(4):
            for o in range(2):
                w3pack[tau, 32 * b:32 * b + 32, 8 * tau + 4 * o + b] = W3_small[:, o]
    w3s = np.ascontiguousarray(w3pack.transpose(1, 0, 2).reshape(128, 2048))

    bc1 = np.zeros((128, 1), np.float32)
    bc2 = np.zeros((128, 1), np.float32)
    bc3 = np.zeros((128, 1), np.float32)
    for b in range(4):
        bc1[32 * b:32 * b + 32, 0] = b0.astype(np.float32) / 256.0 + CTIE
        bc2[32 * b:32 * b + 32, 0] = b1.astype(np.float32) / 256.0 + CTIE
    for tau in range(16):
        for o in range(2):
            for b in range(4):
                bc3[8 * tau + 4 * o + b, 0] = float(b_out[o]) / 256.0 + CTIE
    return w1s, w2s, w3s, bc1, bc2, bc3


def _prep_x_core(xs):
    """[S, 32] f32 -> (xhi [NT//4,128,2048] fp16, xlo same-shape fp8 bytes)."""
    import ml_dtypes
    xd = xs.reshape(NT, 4, 512, C).transpose(0, 1, 3, 2).reshape(NT, 128, 512)
    xh = xd.astype(np.float16)
    xl = (xd - xh.astype(np.float32)).astype(ml_dtypes.float8_e4m3fn)
    xh4 = xh.reshape(NT // 4, 4, 128, 512).transpose(0, 2, 1, 3).reshape(
        NT // 4, 128, 2048)
    xl4 = xl.reshape(NT // 4, 4, 128, 512).transpose(0, 2, 1, 3).reshape(
        NT // 4, 128, 2048)
    return np.ascontiguousarray(xh4), np.ascontiguousarray(xl4).view(np.uint8)


def kernel(x, w0, b0, w1, b1, w_out, b_out):
    x = np.ascontiguousarray(np.asarray(x, np.float32))
    w1s, w2s, w3s, bc1, bc2, bc3 = _prep_weights(
        np.asarray(w0), np.asarray(b0), np.asarray(w1), np.asarray(b1),
        np.asarray(w_out), np.asarray(b_out))

    nc = _get_graph()

    in_maps = []
    for i in range(NCORES):
        xh4, xl4 = _prep_x_core(x[i * S:(i + 1) * S])
        in_maps.append({"xhi": xh4, "xlo": xl4, "w1s": w1s, "w2s": w2s,
                        "w3s": w3s, "bc1": bc1, "bc2": bc2, "bc3": bc3})

    res = run_bass_kernel_spmd(nc, in_maps, list(range(NCORES))).results

    mu = np.empty(B, np.float32)
    ls = np.empty(B, np.float32)
    sc = np.empty(B, np.float32)
    for i in range(NCORES):
        # outa[pack, 8 tau + 4 o + b, f'] = raw(row = 2048(16 pack+tau)+512 b+f', o)
        a = np.asarray(res[i]["outa"], np.float32).reshape(NPACK, 16, 2, 4, 512)
        bb = np.asarray(res[i]["outb"], np.float32).reshape(NPACK, 16, 2, 4, 512)
        sl = slice(i * S, (i + 1) * S)
        mu[sl] = a[:, :, 0].reshape(S)
        ls[sl] = a[:, :, 1].reshape(S)
        sc[sl] = bb[:, :, 1].reshape(S)
    return mu, sc, ls


if __name__ == "__main__":
    rng = np.random.default_rng(0)
    x = rng.standard_normal((B, C)).astype(np.float32)
    w0 = np.round(rng.standard_normal((C, C)) * 13).astype(np.float32)
    b0 = np.round(rng.standard_normal(C) * 3000).astype(np.float32)
    w1 = np.round(rng.standard_normal((C, C)) * 13).astype(np.float32)
    b1 = np.round(rng.standard_normal(C) * 3000).astype(np.float32)
    w_out = np.round(rng.standard_normal((2, C)) * 13).astype(np.float32)
    b_out = np.round(rng.standard_normal(2) * 3000).astype(np.float32)
    out = kernel(x, w0, b0, w1, b1, w_out, b_out)
    print([o.shape for o in out], [float(np.abs(o).mean()) for o in out])


# revision 29
# speedup vs baseline: 1.0235x; 1.0235x over previous
"""Trainium2 Bass kernel for nn_ArmInt_19911468384433 (dense_mlp, 8 cores).

Data-parallel: x [2097152, 32] sharded by rows across 8 NeuronCores; tiny
32x32 weights folded/replicated. All math (3 integer-MLP layers with
emulated fixed-point rounding, exp/clip head) runs on device. Host does
layout permutation (shard + transpose-pack of input, fp16 hi/lo split,
inverse reshape of output) and weight folding only.

Device algorithm per core (S = 262144 rows = 128 tiles of 2048 rows):
  x pre-packed on host as xd*[t, 32b+c, f'] = x[2048 t + 512 b + f', c],
  split x = xhi (fp16) + xlo (fp8e4m3): ~15-bit combined mantissa, 3B/elem
  input DMA; boundary-flip rate stays within the 2e-2 error budget.
  Per tile:
    mm1 : ps1 = W1s.T @ xhi + W1s.T @ xlo   (fp16 stationary; fp8 moving
          for the lo half — mixed-dtype matmul, PSUM f32 accumulate)
    ep1 : t = Relu(ps1 + bc1) (ACT, bias AP)  then h1 = rne(t) via DVE
          fused ts (+2^23, -2^23) -> fp16   [or the all-DVE form below]
    mm2 : ps2 = W2s.T @ h1  (fp16);  ep2 likewise -> h2
    mm3 : ps3pack += W3s_tau.T @ h2  (16 tiles accumulate into one bank)
  Every 4th layer-slot uses the all-DVE form to balance engines:
    u = ts(ps + bc, + 2^23) ; h = ts(u - 2^23, max 0) -> fp16
  Per 16-tile pack (DVE except Exp):
    t3 = ps3pack + bc3 ; y = rne(t3) ; outA = y/256
    e = Exp(y/256 - 4) (ACT) ; outB = clip(e, e^-4.6, e^5)

rne(v + 2^-9) == trunc-round-half-away-from-zero on the 1/256-granular
values here; verified vs reference at rel err ~5e-3 (fp32 order noise).
"""
import sys

sys.path.insert(0, "/opt/trn_rl_repo")

from contextlib import ExitStack

import numpy as np

import concourse.bacc as bacc
import concourse.bass as bass
import concourse.tile as tile
from concourse import mybir
from concourse.bass_utils import run_bass_kernel_spmd

F32 = mybir.dt.float32
F16 = mybir.dt.float16
F8 = mybir.dt.float8e4
AF = mybir.ActivationFunctionType
ALU = mybir.AluOpType

B = 2097152
C = 32
NCORES = 8
S = B // NCORES            # 262144 rows per core
NT = S // 2048             # 128 tiles per core
NPACK = NT // 16           # 8 packs per core
C2 = float(2.0 ** 23)
CTIE = float(2.0 ** -9)

_compiled = {}


def _layer_ep(nc, pools, ps, bc_sb, bcm_sb, out_dt, dve_form, tagp):
    """PSUM -> relu(rne(v + bc)) -> SBUF tile (out_dt)."""
    tpool, hpool = pools
    if not dve_form:
        t = tpool.tile([128, 512], F32, tag=tagp + "t", name="t")
        nc.scalar.activation(t, ps, AF.Relu, bias=bc_sb, scale=1.0)
        h = hpool.tile([128, 512], out_dt, tag=tagp + "h", name="h")
        nc.vector.tensor_scalar(h, t, C2, C2, ALU.add, ALU.subtract)
    else:
        u = tpool.tile([128, 512], F32, tag=tagp + "t", name="u")
        nc.vector.tensor_scalar(u, ps, bc_sb, C2, ALU.add, ALU.add)
        h = hpool.tile([128, 512], out_dt, tag=tagp + "h", name="h")
        nc.vector.tensor_scalar(h, u, C2, 0.0, ALU.subtract, ALU.max)
    return h


def _build_graph():
    nc = bacc.Bacc("TRN2", target_bir_lowering=False, debug=False)
    # x split: hi fp16 + lo fp8e4m3 (exact-enough 15-bit x; mixed-dtype
    # matmul fp16 stationary x fp8 moving verified exact on HW).
    # 4 tiles per DMA group: xhi [128, 4*512] fp16, xlo [128, 4*512] fp8.
    xhi = nc.declare_dram_parameter("xhi", [NT // 4, 128, 2048], F16, isOutput=False)
    xlo = nc.declare_dram_parameter("xlo", [NT // 4, 128, 2048], F8, isOutput=False)
    w1s = nc.declare_dram_parameter("w1s", [128, 128], F16, isOutput=False)
    w2s = nc.declare_dram_parameter("w2s", [128, 128], F16, isOutput=False)
    w3s = nc.declare_dram_parameter("w3s", [128, 2048], F16, isOutput=False)
    bc1 = nc.declare_dram_parameter("bc1", [128, 1], F32, isOutput=False)
    bc2 = nc.declare_dram_parameter("bc2", [128, 1], F32, isOutput=False)
    bc3 = nc.declare_dram_parameter("bc3", [128, 1], F32, isOutput=False)
    outa = nc.declare_dram_parameter("outa", [NPACK, 128, 512], F16, isOutput=True)
    outb = nc.declare_dram_parameter("outb", [NPACK, 128, 512], F16, isOutput=True)

    with ExitStack() as ctx:
        tc = ctx.enter_context(tile.TileContext(nc))
        consts = ctx.enter_context(tc.tile_pool(name="consts", bufs=1))
        xpool = ctx.enter_context(tc.tile_pool(name="xpool", bufs=5))
        tpool = ctx.enter_context(tc.tile_pool(name="tpool", bufs=4))
        hpool = ctx.enter_context(tc.tile_pool(name="hpool", bufs=4))
        opool = ctx.enter_context(tc.tile_pool(name="opool", bufs=3))
        ps1p = ctx.enter_context(tc.tile_pool(name="ps1p", bufs=2, space="PSUM"))
        ps2p = ctx.enter_context(tc.tile_pool(name="ps2p", bufs=2, space="PSUM"))
        ps3p = ctx.enter_context(tc.tile_pool(name="ps3p", bufs=2, space="PSUM"))

        w1_sb = consts.tile([128, 128], F16, tag="w1", name="w1_sb")
        nc.gpsimd.dma_start(out=w1_sb, in_=w1s[:])
        w2_sb = consts.tile([128, 128], F16, tag="w2", name="w2_sb")
        nc.gpsimd.dma_start(out=w2_sb, in_=w2s[:])
        w3_sb = consts.tile([128, 2048], F16, tag="w3", name="w3_sb")
        nc.gpsimd.dma_start(out=w3_sb, in_=w3s[:])
        bc1_sb = consts.tile([128, 1], F32, tag="bc1", name="bc1_sb")
        nc.gpsimd.dma_start(out=bc1_sb, in_=bc1[:])
        bc2_sb = consts.tile([128, 1], F32, tag="bc2", name="bc2_sb")
        nc.gpsimd.dma_start(out=bc2_sb, in_=bc2[:])
        bc3_sb = consts.tile([128, 1], F32, tag="bc3", name="bc3_sb")
        nc.gpsimd.dma_start(out=bc3_sb, in_=bc3[:])
        bm4_sb = consts.tile([128, 1], F32, tag="bm4", name="bm4_sb")
        nc.vector.memset(bm4_sb, -4.0)
        c2_sb = consts.tile([128, 1], F32, tag="c2c", name="c2_sb")
        nc.vector.memset(c2_sb, C2)

        for pack in range(NPACK):
            ps3 = ps3p.tile([128, 512], F32, tag="ps3", name="ps3")
            for tau in range(16):
                t = pack * 16 + tau
                if tau % 4 == 0:
                    xh4 = xpool.tile([128, 2048], F16, tag="xh4", name="xh4")
                    xl4 = xpool.tile([128, 2048], F8, tag="xl4", name="xl4")
                    nc.sync.dma_start(out=xh4, in_=xhi[t // 4])
                    nc.gpsimd.dma_start(out=xl4, in_=xlo[t // 4])
                if tau % 2 == 0:
                    t1b = tpool.tile([128, 1024], F32, tag="t1b", name="t1b")
                    h1b = hpool.tile([128, 1024], F16, tag="h1b", name="h1b")
                off = 512 * (tau % 4)

                # layer 1: per-tile 1-bank PSUM; evacs into halves of t1b,
                # one batched rne per 2 tiles
                ps1 = ps1p.tile([128, 512], F32, tag="ps1", name="ps1")
                nc.tensor.matmul(ps1, w1_sb, xh4[:, off:off + 512],
                                 start=True, stop=False)
                nc.tensor.matmul(ps1, w1_sb, xl4[:, off:off + 512],
                                 start=False, stop=True)
                half = slice(512 * (tau % 2), 512 * (tau % 2) + 512)
                nc.scalar.activation(t1b[:, half], ps1, AF.Relu, bias=bc1_sb,
                                     scale=1.0)
                if tau % 2 == 1:
                    nc.vector.tensor_scalar(h1b, t1b, C2, C2, ALU.add,
                                            ALU.subtract)

                # layer 2 for the two tiles of the group (after h1b ready).
                # Pattern over 4 tiles: [A A D D] — A-pairs share one batched
                # rne over [128, 1024]; D uses the all-DVE per-tile form.
                if tau % 2 == 1:
                    act_pair = (t % 4) == 1
                    if act_pair:
                        t2b = tpool.tile([128, 1024], F32, tag="t2b", name="t2b")
                        h2b = hpool.tile([128, 1024], F16, tag="h2b", name="h2b")
                    h2s = []
                    for k, tprev in enumerate((t - 1, t)):
                        h1 = h1b[:, 512 * (tprev % 2):512 * (tprev % 2) + 512]
                        ps2 = ps2p.tile([128, 512], F32, tag="ps2", name="ps2")
                        nc.tensor.matmul(ps2, w2_sb, h1, start=True, stop=True)
                        if act_pair:
                            nc.scalar.activation(t2b[:, 512 * k:512 * k + 512],
                                                 ps2, AF.Relu, bias=bc2_sb,
                                                 scale=1.0)
                            h2s.append(None)
                        else:
                            h2s.append(_layer_ep(nc, (tpool, hpool), ps2,
                                                 bc2_sb, bm4_sb, F16,
                                                 dve_form=True, tagp="l2"))
                    if act_pair:
                        nc.vector.tensor_scalar(h2b, t2b, C2, C2, ALU.add,
                                                ALU.subtract)
                        h2s = [h2b[:, 0:512], h2b[:, 512:1024]]
                    for k, tprev in enumerate((t - 1, t)):
                        tau_p = tprev - pack * 16
                        nc.tensor.matmul(
                            ps3, w3_sb[:, 128 * tau_p:128 * (tau_p + 1)],
                            h2s[k], start=(tau_p == 0), stop=(tau_p == 15))

            t3 = tpool.tile([128, 512], F32, tag="t3", name="t3")
            nc.vector.tensor_scalar(t3, ps3, bc3_sb, C2, ALU.add, ALU.add)
            y = tpool.tile([128, 512], F32, tag="y", name="y")
            nc.vector.tensor_scalar(y, t3, C2, None, ALU.subtract)
            oa = opool.tile([128, 512], F16, tag="oa", name="oa")
            nc.vector.tensor_scalar(oa, y, 1.0 / 256.0, None, ALU.mult)
            e = tpool.tile([128, 512], F32, tag="e", name="e")
            nc.scalar.activation(e, y, AF.Exp, bias=bm4_sb, scale=1.0 / 256.0)
            ob = opool.tile([128, 512], F16, tag="ob", name="ob")
            nc.vector.tensor_scalar(ob, e, float(np.exp(5.0)),
                                    float(np.exp(-4.6)), ALU.min, ALU.max)
            nc.sync.dma_start(out=outa[pack], in_=oa)
            nc.sync.dma_start(out=outb[pack], in_=ob)

    nc.compile()
    return nc


def _get_graph():
    if "nc" not in _compiled:
        _compiled["nc"] = _build_graph()
    return _compiled["nc"]


def _prep_weights(w0, b0, w1, b1, w_out, b_out):
    eye = np.eye(C, dtype=np.float32)
    W0s = (w0.T.astype(np.float32) + 256.0 * eye).astype(np.float16)
    W1s_small = ((w1.T.astype(np.float32) + 256.0 * eye) / 256.0).astype(np.float16)
    W3_small = (w_out.T.astype(np.float32) / 256.0).astype(np.float16)  # [32, 2]

    w1s = np.zeros((128, 128), np.float16)
    w2s = np.zeros((128, 128), np.float16)
    for b in range(4):
        w1s[32 * b:32 * b + 32, 32 * b:32 * b + 32] = W0s
        w2s[32 * b:32 * b + 32, 32 * b:32 * b + 32] = W1s_small

    # mm3 stationary for inner-loop index tau: out partition m = 8 tau + 4 o + b
    w3pack = np.zeros((16, 128, 128), np.float16)
    for tau in range(16):
        for b in range(4):
            for o in range(2):
                w3pack[tau, 32 * b:32 * b + 32, 8 * tau + 4 * o + b] = W3_small[:, o]
    w3s = np.ascontiguousarray(w3pack.transpose(1, 0, 2).reshape(128, 2048))

    bc1 = np.zeros((128, 1), np.float32)
    bc2 = np.zeros((128, 1), np.float32)
    bc3 = np.zeros((128, 1), np.float32)
    for b in range(4):
        bc1[32 * b:32 * b + 32, 0] = b0.astype(np.float32) / 256.0 + CTIE
        bc2[32 * b:32 * b + 32, 0] = b1.astype(np.float32) / 256.0 + CTIE
    for tau in range(16):
        for o in range(2):
            for b in range(4):
                bc3[8 * tau + 4 * o + b, 0] = float(b_out[o]) / 256.0 + CTIE
    return w1s, w2s, w3s, bc1, bc2, bc3


def _prep_x_core(xs):
    """[S, 32] f32 -> (xhi [NT//4,128,2048] fp16, xlo same-shape fp8 bytes)."""
    import ml_dtypes
    xd = xs.reshape(NT, 4, 512, C).transpose(0, 1, 3, 2).reshape(NT, 128, 512)
    xh = xd.astype(np.float16)
    xl = (xd - xh.astype(np.float32)).astype(ml_dtypes.float8_e4m3fn)
    xh4 = xh.reshape(NT // 4, 4, 128, 512).transpose(0, 2, 1, 3).reshape(
        NT // 4, 128, 2048)
    xl4 = xl.reshape(NT // 4, 4, 128, 512).transpose(0, 2, 1, 3).reshape(
        NT // 4, 128, 2048)
    return np.ascontiguousarray(xh4), np.ascontiguousarray(xl4).view(np.uint8)


def kernel(x, w0, b0, w1, b1, w_out, b_out):
    x = np.ascontiguousarray(np.asarray(x, np.float32))
    w1s, w2s, w3s, bc1, bc2, bc3 = _prep_weights(
        np.asarray(w0), np.asarray(b0), np.asarray(w1), np.asarray(b1),
        np.asarray(w_out), np.asarray(b_out))

    nc = _get_graph()

    in_maps = []
    for i in range(NCORES):
        xh4, xl4 = _prep_x_core(x[i * S:(i + 1) * S])
        in_maps.append({"xhi": xh4, "xlo": xl4, "w1s": w1s, "w2s": w2s,
                        "w3s": w3s, "bc1": bc1, "bc2": bc2, "bc3": bc3})

    res = run_bass_kernel_spmd(nc, in_maps, list(range(NCORES))).results

    mu = np.empty(B, np.float32)
    ls = np.empty(B, np.float32)
    sc = np.empty(B, np.float32)
    for i in range(NCORES):
        # outa[pack, 8 tau + 4 o + b, f'] = raw(row = 2048(16 pack+tau)+512 b+f', o)
        a = np.asarray(res[i]["outa"], np.float32).reshape(NPACK, 16, 2, 4, 512)
        bb = np.asarray(res[i]["outb"], np.float32).reshape(NPACK, 16, 2, 4, 512)
        sl = slice(i * S, (i + 1) * S)
        mu[sl] = a[:, :, 0].reshape(S)
        ls[sl] = a[:, :, 1].reshape(S)
        sc[sl] = bb[:, :, 1].reshape(S)
    return mu, sc, ls


if __name__ == "__main__":
    rng = np.random.default_rng(0)
    x = rng.standard_normal((B, C)).astype(np.float32)
    w0 = np.round(rng.standard_normal((C, C)) * 13).astype(np.float32)
    b0 = np.round(rng.standard_normal(C) * 3000).astype(np.float32)
    w1 = np.round(rng.standard_normal((C, C)) * 13).astype(np.float32)
    b1 = np.round(rng.standard_normal(C) * 3000).astype(np.float32)
    w_out = np.round(rng.standard_normal((2, C)) * 13).astype(np.float32)
    b_out = np.round(rng.standard_normal(2) * 3000).astype(np.float32)
    out = kernel(x, w0, b0, w1, b1, w_out, b_out)
    print([o.shape for o in out], [float(np.abs(o).mean()) for o in out])
